# revision 8
# baseline (speedup 1.0000x reference)
"""Trainium2 Bass kernel for nn_CachedCompressedLinear.

out[16, 11008] = x[16, 4096] @ ((w_q - 128) * scale).T + bias

Sharding: column-parallel over 8 NeuronCores; each core computes a
[16, 1376] slice of the output (11008 = 8 * 1376).

The kernel is HBM-bound: the weight codes ship as 1 byte each
(w_q - 128 fits int8), 4x less traffic than the int32 input encoding.
The int8 -> bf16 decode is split DVE (tensor_scalar_add, ~0.88us per
k-tile) / ACT (activation Copy, ~1.44us) so both engines stay under
the ~17us DMA floor.  GpSimd gets NO tensor work: its Q7 software
loops hog the SBUF ports it shares with DVE and stall DVE for the
entire op (measured: each 20us GpSimd decode blocked DVE ~20us).
GpSimd only issues const/output DMAs (DGE trigger, harmless).

scale folds into x on the host (bf16 x, ~1.6e-3 rel err vs 2e-2
gate); bias folds into PSUM via one K=1 ones-row matmul per output
chunk, so the epilogue is a bare PSUM -> SBUF copy + DMA out.
"""

import sys

if "/opt/trn_rl_repo" not in sys.path:
    sys.path.insert(0, "/opt/trn_rl_repo")

import numpy as np
import ml_dtypes

IN_F = 4096
OUT_F = 11008
BATCH = 16
N_CORES = 8
O_PER = 1376  # out_features per core (11008 = 8 * 1376)
K_TILES = IN_F // 128  # 32
M = BATCH  # stationary columns (bf16 x, no hi/lo split)
CHUNKS = [(0, 512), (512, 512), (1024, 352)]  # o-chunks within 1376
PAIR = 2  # k-tiles per weight DMA

# per-PAIR decode engine: D=DVE, A=ACT; 10:6 ratio matches the
# measured pair-fused decode times (~1.8us DVE vs ~2.9us ACT).
PAIR_PAT = "DADADADDADADDADD"
WARMUP_MM = 6  # dummy matmuls to ramp the PE pstate before the stream

_BUILT = None


def _build():
    """Build the (SPMD, per-core) Bass program once."""
    import concourse.bass as bass
    import concourse.tile as tile
    from concourse import bacc, mybir

    dt = mybir.dt
    nc = bacc.Bacc("TRN2", target_bir_lowering=False, debug=False)

    w8 = nc.dram_tensor("w8", [128, K_TILES * O_PER], dt.int8, kind="ExternalInput")
    xt = nc.dram_tensor(
        "xt", [128, (K_TILES + 1) * M], dt.bfloat16, kind="ExternalInput"
    )
    bias_bf = nc.dram_tensor("bias_bf", [1, O_PER], dt.bfloat16, kind="ExternalInput")
    out = nc.dram_tensor("out", [BATCH, O_PER], dt.float32, kind="ExternalOutput")

    with tile.TileContext(nc) as tc:
        with (
            tc.tile_pool(name="consts", bufs=1) as consts,
            tc.tile_pool(name="w8p", bufs=6) as w8p,
            tc.tile_pool(name="wbf", bufs=6) as wbfp,
            tc.tile_pool(name="psum", bufs=1, space=bass.MemorySpace.PSUM) as psump,
            tc.tile_pool(name="outp", bufs=1) as outp,
        ):
            x_sb = consts.tile([128, (K_TILES + 1) * M], dt.bfloat16)
            nc.sync.dma_start(x_sb[:], xt[:])
            bias_sb = consts.tile([1, O_PER], dt.bfloat16)
            nc.sync.dma_start(bias_sb[:], bias_bf[:])

            psums = [
                psump.tile([M, w], dt.float32, name=f"ps{i}", tag=f"ps{i}")
                for i, (_, w) in enumerate(CHUNKS)
            ]
            ones_blk = x_sb[0:1, K_TILES * M : K_TILES * M + M]  # [1, 16]

            # ramp the PE pstate while the first weight pairs are in
            # flight: dummy K=1 matmuls into a scratch PSUM bank
            ps_warm = psump.tile([M, 512], dt.float32, name="ps_warm")
            for _ in range(WARMUP_MM):
                nc.tensor.matmul(
                    ps_warm[:, :], ones_blk, bias_sb[0:1, 0:512],
                    start=True, stop=True,
                )

            for g in range(K_TILES // PAIR):
                w8_t = w8p.tile([128, PAIR * O_PER], dt.int8, tag="w8_t")
                nc.sync.dma_start(
                    w8_t[:], w8[:][:, g * PAIR * O_PER : (g + 1) * PAIR * O_PER]
                )
                wb_t = wbfp.tile([128, PAIR * O_PER], dt.bfloat16, tag="wb_t")
                if PAIR_PAT[g] == "D":
                    nc.vector.tensor_scalar_add(wb_t[:], w8_t[:], 0.0)
                else:
                    nc.scalar.activation(
                        wb_t[:], w8_t[:], mybir.ActivationFunctionType.Copy
                    )
                for j in range(PAIR):
                    k = g * PAIR + j
                    for i, (o, w) in enumerate(CHUNKS):
                        nc.tensor.matmul(
                            psums[i][:, :],
                            x_sb[:, k * M : (k + 1) * M],
                            wb_t[:, j * O_PER + o : j * O_PER + o + w],
                            start=(k == 0),
                            stop=(k == K_TILES - 1),
                        )
                    if k == 0:
                        # fold bias into PSUM: ones stationary x bias moving
                        for i, (o, w) in enumerate(CHUNKS):
                            nc.tensor.matmul(
                                psums[i][:, :],
                                ones_blk,
                                bias_sb[0:1, o : o + w],
                                start=False,
                                stop=False,
                            )

            out_q = [nc.gpsimd, nc.sync, nc.scalar]
            for i, (o, w) in enumerate(CHUNKS):
                ob = outp.tile([BATCH, w], dt.float32, name=f"ob{i}")
                if i == 1:
                    nc.scalar.activation(
                        ob[:], psums[i][:, :], mybir.ActivationFunctionType.Copy
                    )
                else:
                    nc.vector.tensor_copy(ob[:], psums[i][:, :])
                out_q[i].dma_start(out[:][:, o : o + w], ob[:])

    nc.compile()
    return nc


def _get_built():
    global _BUILT
    if _BUILT is None:
        _BUILT = _build()
    return _BUILT


def make_in_maps(x, w_q, scale, bias):
    """Host-side shard + layout prep. Returns per-core input dicts."""
    x = np.asarray(x, dtype=np.float32)
    w_q = np.asarray(w_q, dtype=np.int32)
    scale = np.asarray(scale, dtype=np.float32)
    bias = np.asarray(bias, dtype=np.float32)

    s = float(scale.reshape(-1)[0])
    xsT = np.ascontiguousarray((x * s).T).astype(ml_dtypes.bfloat16)  # [4096, 16]
    # SBUF layout [128, K_TILES*M]: partition p holds, for each k-tile t,
    # the stationary block row (t*128 + p); plus a ones block for bias.
    xt = np.zeros((128, (K_TILES + 1) * M), dtype=ml_dtypes.bfloat16)
    xt[:, : K_TILES * M] = (
        xsT.reshape(K_TILES, 128, M).transpose(1, 0, 2).reshape(128, K_TILES * M)
    )
    xt[0, K_TILES * M : K_TILES * M + M] = 1.0

    w8_full = (w_q.astype(np.int16) - 128).astype(np.int8)  # [11008, 4096]

    in_maps = []
    for c in range(N_CORES):
        sl = w8_full[c * O_PER : (c + 1) * O_PER].T  # [4096, 1376] int8
        w8_c = np.ascontiguousarray(
            sl.reshape(K_TILES, 128, O_PER).transpose(1, 0, 2).reshape(128, -1)
        )
        bias_c = np.ascontiguousarray(
            bias[c * O_PER : (c + 1) * O_PER].reshape(1, O_PER)
        ).astype(ml_dtypes.bfloat16)
        in_maps.append({"w8": w8_c, "xt": xt, "bias_bf": bias_c})
    return in_maps


def run(inputs, trace=False):
    """Run on the 8 NeuronCores. Returns (full_output, BassKernelResults)."""
    from concourse.bass_utils import run_bass_kernel_spmd

    in_maps = make_in_maps(**inputs)
    nc = _get_built()
    res = run_bass_kernel_spmd(nc, in_maps, list(range(N_CORES)), trace=trace)
    parts = [np.asarray(res.results[c]["out"]) for c in range(N_CORES)]
    full = np.concatenate(parts, axis=1)[:, :OUT_F].astype(np.float32)
    return full, res


def kernel(**inputs) -> np.ndarray:
    full, _ = run(inputs, trace=False)
    return full


# revision 9
# speedup vs baseline: 1.0939x; 1.0939x over previous
"""Trainium2 Bass kernel for nn_CachedCompressedLinear.

out[16, 11008] = x[16, 4096] @ ((w_q - 128) * scale).T + bias

Sharding: column-parallel over 8 NeuronCores; each core computes a
[16, 1376] slice of the output (11008 = 8 * 1376).

The kernel is HBM-bound: the weight codes ship as 1 byte each
(w_q - 128 fits int8), 4x less traffic than the int32 input encoding.
The int8 -> bf16 decode is split DVE (tensor_scalar_add, ~0.88us per
k-tile) / ACT (activation Copy, ~1.44us) so both engines stay under
the ~17us DMA floor.  GpSimd gets NO tensor work: its Q7 software
loops hog the SBUF ports it shares with DVE and stall DVE for the
entire op (measured: each 20us GpSimd decode blocked DVE ~20us).
GpSimd only issues const/output DMAs (DGE trigger, harmless).

scale folds into x on the host (bf16 x, ~1.6e-3 rel err vs 2e-2
gate); bias folds into PSUM via one K=1 ones-row matmul per output
chunk, so the epilogue is a bare PSUM -> SBUF copy + DMA out.
"""

import sys

if "/opt/trn_rl_repo" not in sys.path:
    sys.path.insert(0, "/opt/trn_rl_repo")

import numpy as np
import ml_dtypes

IN_F = 4096
OUT_F = 11008
BATCH = 16
N_CORES = 8
O_PER = 1376  # out_features per core (11008 = 8 * 1376)
K_TILES = IN_F // 128  # 32
M = BATCH  # stationary columns (bf16 x, no hi/lo split)
CHUNKS = [(0, 512), (512, 512), (1024, 352)]  # o-chunks within 1376
PAIR = 2  # k-tiles per weight DMA

# per-k-tile decode engine: D=DVE, A=ACT; 20:12 matches measured
# per-tile decode times (1052ns DVE vs 1729ns ACT); A tiles sit
# second-in-pair so their higher latency gets an extra half-pair of
# slack before the PE needs them.
DECODE_PAT = "DADADADD" * 4

_BUILT = None


def _build():
    """Build the (SPMD, per-core) Bass program once."""
    import concourse.bass as bass
    import concourse.tile as tile
    from concourse import bacc, mybir

    dt = mybir.dt
    nc = bacc.Bacc("TRN2", target_bir_lowering=False, debug=False)

    w8 = nc.dram_tensor("w8", [128, K_TILES * O_PER], dt.int8, kind="ExternalInput")
    xt = nc.dram_tensor(
        "xt", [128, (K_TILES + 1) * M], dt.bfloat16, kind="ExternalInput"
    )
    bias_bf = nc.dram_tensor("bias_bf", [1, O_PER], dt.bfloat16, kind="ExternalInput")
    out = nc.dram_tensor("out", [BATCH, O_PER], dt.float32, kind="ExternalOutput")

    with tile.TileContext(nc) as tc:
        with (
            tc.tile_pool(name="consts", bufs=1) as consts,
            tc.tile_pool(name="w8p", bufs=6) as w8p,
            tc.tile_pool(name="wbf", bufs=8) as wbfp,
            tc.tile_pool(name="psum", bufs=1, space=bass.MemorySpace.PSUM) as psump,
            tc.tile_pool(name="outp", bufs=1) as outp,
        ):
            x_sb = consts.tile([128, (K_TILES + 1) * M], dt.bfloat16)
            nc.gpsimd.dma_start(x_sb[:], xt[:])
            bias_sb = consts.tile([1, O_PER], dt.bfloat16)
            nc.gpsimd.dma_start(bias_sb[:], bias_bf[:])

            psums = [
                psump.tile([M, w], dt.float32, name=f"ps{i}", tag=f"ps{i}")
                for i, (_, w) in enumerate(CHUNKS)
            ]
            ones_blk = x_sb[0:1, K_TILES * M : K_TILES * M + M]  # [1, 16]

            for g in range(K_TILES // PAIR):
                w8_t = w8p.tile([128, PAIR * O_PER], dt.int8, tag="w8_t")
                nc.sync.dma_start(
                    w8_t[:], w8[:][:, g * PAIR * O_PER : (g + 1) * PAIR * O_PER]
                )
                for j in range(PAIR):
                    k = g * PAIR + j
                    wb_t = wbfp.tile([128, O_PER], dt.bfloat16, tag="wb_t")
                    src_ap = w8_t[:, j * O_PER : (j + 1) * O_PER]
                    if DECODE_PAT[k] == "D":
                        nc.vector.tensor_scalar_add(wb_t[:], src_ap, 0.0)
                    else:
                        nc.scalar.activation(
                            wb_t[:], src_ap, mybir.ActivationFunctionType.Copy
                        )
                    for i, (o, w) in enumerate(CHUNKS):
                        nc.tensor.matmul(
                            psums[i][:, :],
                            x_sb[:, k * M : (k + 1) * M],
                            wb_t[:, o : o + w],
                            start=(k == 0),
                            stop=(k == K_TILES - 1),
                        )
                    if k == 0:
                        # fold bias into PSUM: ones stationary x bias moving
                        for i, (o, w) in enumerate(CHUNKS):
                            nc.tensor.matmul(
                                psums[i][:, :],
                                ones_blk,
                                bias_sb[0:1, o : o + w],
                                start=False,
                                stop=False,
                            )

            out_q = [nc.gpsimd, nc.sync, nc.scalar]
            for i, (o, w) in enumerate(CHUNKS):
                ob = outp.tile([BATCH, w], dt.float32, name=f"ob{i}")
                if i == 1:
                    nc.scalar.activation(
                        ob[:], psums[i][:, :], mybir.ActivationFunctionType.Copy
                    )
                else:
                    nc.vector.tensor_copy(ob[:], psums[i][:, :])
                out_q[i].dma_start(out[:][:, o : o + w], ob[:])

    nc.compile()
    return nc


def _get_built():
    global _BUILT
    if _BUILT is None:
        _BUILT = _build()
    return _BUILT


def make_in_maps(x, w_q, scale, bias):
    """Host-side shard + layout prep. Returns per-core input dicts."""
    x = np.asarray(x, dtype=np.float32)
    w_q = np.asarray(w_q, dtype=np.int32)
    scale = np.asarray(scale, dtype=np.float32)
    bias = np.asarray(bias, dtype=np.float32)

    s = float(scale.reshape(-1)[0])
    xsT = np.ascontiguousarray((x * s).T).astype(ml_dtypes.bfloat16)  # [4096, 16]
    # SBUF layout [128, K_TILES*M]: partition p holds, for each k-tile t,
    # the stationary block row (t*128 + p); plus a ones block for bias.
    xt = np.zeros((128, (K_TILES + 1) * M), dtype=ml_dtypes.bfloat16)
    xt[:, : K_TILES * M] = (
        xsT.reshape(K_TILES, 128, M).transpose(1, 0, 2).reshape(128, K_TILES * M)
    )
    xt[0, K_TILES * M : K_TILES * M + M] = 1.0

    w8_full = (w_q.astype(np.int16) - 128).astype(np.int8)  # [11008, 4096]

    in_maps = []
    for c in range(N_CORES):
        sl = w8_full[c * O_PER : (c + 1) * O_PER].T  # [4096, 1376] int8
        w8_c = np.ascontiguousarray(
            sl.reshape(K_TILES, 128, O_PER).transpose(1, 0, 2).reshape(128, -1)
        )
        bias_c = np.ascontiguousarray(
            bias[c * O_PER : (c + 1) * O_PER].reshape(1, O_PER)
        ).astype(ml_dtypes.bfloat16)
        in_maps.append({"w8": w8_c, "xt": xt, "bias_bf": bias_c})
    return in_maps


def run(inputs, trace=False):
    """Run on the 8 NeuronCores. Returns (full_output, BassKernelResults)."""
    from concourse.bass_utils import run_bass_kernel_spmd

    in_maps = make_in_maps(**inputs)
    nc = _get_built()
    res = run_bass_kernel_spmd(nc, in_maps, list(range(N_CORES)), trace=trace)
    parts = [np.asarray(res.results[c]["out"]) for c in range(N_CORES)]
    full = np.concatenate(parts, axis=1)[:, :OUT_F].astype(np.float32)
    return full, res


def kernel(**inputs) -> np.ndarray:
    full, _ = run(inputs, trace=False)
    return full


# revision 10
# speedup vs baseline: 1.1142x; 1.0186x over previous
"""Trainium2 Bass kernel for nn_CachedCompressedLinear.

out[16, 11008] = x[16, 4096] @ ((w_q - 128) * scale).T + bias

Sharding: column-parallel over 8 NeuronCores; each core computes a
[16, 1376] slice of the output (11008 = 8 * 1376).

The kernel is HBM-bound: the weight codes ship as 1 byte each
(w_q - 128 fits int8), 4x less traffic than the int32 input encoding.
The int8 -> bf16 decode is split DVE (tensor_scalar_add, ~0.88us per
k-tile) / ACT (activation Copy, ~1.44us) so both engines stay under
the ~17us DMA floor.  GpSimd gets NO tensor work: its Q7 software
loops hog the SBUF ports it shares with DVE and stall DVE for the
entire op (measured: each 20us GpSimd decode blocked DVE ~20us).
GpSimd only issues const/output DMAs (DGE trigger, harmless).

scale folds into x on the host (bf16 x, ~1.6e-3 rel err vs 2e-2
gate); bias folds into PSUM via one K=1 ones-row matmul per output
chunk, so the epilogue is a bare PSUM -> SBUF copy + DMA out.
"""

import sys

if "/opt/trn_rl_repo" not in sys.path:
    sys.path.insert(0, "/opt/trn_rl_repo")

import numpy as np
import ml_dtypes

IN_F = 4096
OUT_F = 11008
BATCH = 16
N_CORES = 8
O_PER = 1376  # out_features per core (11008 = 8 * 1376)
K_TILES = IN_F // 128  # 32
M = BATCH  # stationary columns (bf16 x, no hi/lo split)
CHUNKS = [(0, 512), (512, 512), (1024, 352)]  # o-chunks within 1376
PAIR = 2  # k-tiles per weight DMA

# per-k-tile decode engine: D=DVE, A=ACT; 20:12 matches measured
# per-tile decode times (1052ns DVE vs 1729ns ACT); A tiles sit
# second-in-pair so their higher latency gets an extra half-pair of
# slack before the PE needs them.
DECODE_PAT = "DADADADD" * 4

_BUILT = None


def _build():
    """Build the (SPMD, per-core) Bass program once."""
    import concourse.bass as bass
    import concourse.tile as tile
    from concourse import bacc, mybir

    dt = mybir.dt
    nc = bacc.Bacc("TRN2", target_bir_lowering=False, debug=False)

    w8 = nc.dram_tensor("w8", [128, K_TILES * O_PER], dt.int8, kind="ExternalInput")
    xt = nc.dram_tensor(
        "xt", [128, (K_TILES + 1) * M], dt.bfloat16, kind="ExternalInput"
    )
    bias_bf = nc.dram_tensor("bias_bf", [1, O_PER], dt.bfloat16, kind="ExternalInput")
    bias_f32 = nc.dram_tensor(
        "bias_f32", [BATCH, O_PER], dt.float32, kind="ExternalInput"
    )
    out = nc.dram_tensor("out", [BATCH, O_PER], dt.float32, kind="ExternalOutput")

    with tile.TileContext(nc) as tc:
        with (
            tc.tile_pool(name="consts", bufs=1) as consts,
            tc.tile_pool(name="w8p", bufs=6) as w8p,
            tc.tile_pool(name="wbf", bufs=8) as wbfp,
            tc.tile_pool(name="psum", bufs=1, space=bass.MemorySpace.PSUM) as psump,
            tc.tile_pool(name="outp", bufs=1) as outp,
        ):
            x_sb = consts.tile([128, (K_TILES + 1) * M], dt.bfloat16)
            nc.gpsimd.dma_start(x_sb[:], xt[:])
            bias_sb = consts.tile([1, O_PER], dt.bfloat16)
            nc.gpsimd.dma_start(bias_sb[:], bias_bf[:])
            bias_rep = consts.tile([BATCH, O_PER], dt.float32)
            nc.gpsimd.dma_start(bias_rep[:], bias_f32[:])

            psums = [
                psump.tile([M, w], dt.float32, name=f"ps{i}", tag=f"ps{i}")
                for i, (_, w) in enumerate(CHUNKS)
            ]
            ones_blk = x_sb[0:1, K_TILES * M : K_TILES * M + M]  # [1, 16]

            for g in range(K_TILES // PAIR):
                w8_t = w8p.tile([128, PAIR * O_PER], dt.int8, tag="w8_t")
                nc.sync.dma_start(
                    w8_t[:], w8[:][:, g * PAIR * O_PER : (g + 1) * PAIR * O_PER]
                )
                for j in range(PAIR):
                    k = g * PAIR + j
                    wb_t = wbfp.tile([128, O_PER], dt.bfloat16, tag="wb_t")
                    src_ap = w8_t[:, j * O_PER : (j + 1) * O_PER]
                    if k == 0:
                        # split the first decode per chunk so the PE can
                        # start on chunk 0 ~0.7us earlier (pipeline fill)
                        for o, w in CHUNKS:
                            nc.vector.tensor_scalar_add(
                                wb_t[:, o : o + w], src_ap[:, o : o + w], 0.0
                            )
                    elif DECODE_PAT[k] == "D":
                        nc.vector.tensor_scalar_add(wb_t[:], src_ap, 0.0)
                    else:
                        nc.scalar.activation(
                            wb_t[:], src_ap, mybir.ActivationFunctionType.Copy
                        )
                    for i, (o, w) in enumerate(CHUNKS):
                        nc.tensor.matmul(
                            psums[i][:, :],
                            x_sb[:, k * M : (k + 1) * M],
                            wb_t[:, o : o + w],
                            start=(k == 0),
                            stop=(k == K_TILES - 1),
                        )
                    if k == 0:
                        # chunk-2 bias via K=1 matmul; chunks 0/1 get their
                        # bias in the epilogue add (keeps PE row count down)
                        o, w = CHUNKS[2]
                        nc.tensor.matmul(
                            psums[2][:, :],
                            ones_blk,
                            bias_sb[0:1, o : o + w],
                            start=False,
                            stop=False,
                        )

            out_q = [nc.gpsimd, nc.sync, nc.scalar]
            for i, (o, w) in enumerate(CHUNKS):
                ob = outp.tile([BATCH, w], dt.float32, name=f"ob{i}")
                if i == 2:
                    nc.scalar.activation(
                        ob[:], psums[i][:, :], mybir.ActivationFunctionType.Copy
                    )
                else:
                    nc.vector.tensor_add(
                        ob[:], psums[i][:, :], bias_rep[:, o : o + w]
                    )
                out_q[i].dma_start(out[:][:, o : o + w], ob[:])

    nc.compile()
    return nc


def _get_built():
    global _BUILT
    if _BUILT is None:
        _BUILT = _build()
    return _BUILT


def make_in_maps(x, w_q, scale, bias):
    """Host-side shard + layout prep. Returns per-core input dicts."""
    x = np.asarray(x, dtype=np.float32)
    w_q = np.asarray(w_q, dtype=np.int32)
    scale = np.asarray(scale, dtype=np.float32)
    bias = np.asarray(bias, dtype=np.float32)

    s = float(scale.reshape(-1)[0])
    xsT = np.ascontiguousarray((x * s).T).astype(ml_dtypes.bfloat16)  # [4096, 16]
    # SBUF layout [128, K_TILES*M]: partition p holds, for each k-tile t,
    # the stationary block row (t*128 + p); plus a ones block for bias.
    xt = np.zeros((128, (K_TILES + 1) * M), dtype=ml_dtypes.bfloat16)
    xt[:, : K_TILES * M] = (
        xsT.reshape(K_TILES, 128, M).transpose(1, 0, 2).reshape(128, K_TILES * M)
    )
    xt[0, K_TILES * M : K_TILES * M + M] = 1.0

    w8_full = (w_q.astype(np.int16) - 128).astype(np.int8)  # [11008, 4096]

    in_maps = []
    for c in range(N_CORES):
        sl = w8_full[c * O_PER : (c + 1) * O_PER].T  # [4096, 1376] int8
        w8_c = np.ascontiguousarray(
            sl.reshape(K_TILES, 128, O_PER).transpose(1, 0, 2).reshape(128, -1)
        )
        bias_c = np.ascontiguousarray(
            bias[c * O_PER : (c + 1) * O_PER].reshape(1, O_PER)
        )
        bias_rep_c = np.ascontiguousarray(
            np.broadcast_to(bias_c, (BATCH, O_PER))
        ).astype(np.float32)
        in_maps.append({
            "w8": w8_c,
            "xt": xt,
            "bias_bf": bias_c.astype(ml_dtypes.bfloat16),
            "bias_f32": bias_rep_c,
        })
    return in_maps


def run(inputs, trace=False):
    """Run on the 8 NeuronCores. Returns (full_output, BassKernelResults)."""
    from concourse.bass_utils import run_bass_kernel_spmd

    in_maps = make_in_maps(**inputs)
    nc = _get_built()
    res = run_bass_kernel_spmd(nc, in_maps, list(range(N_CORES)), trace=trace)
    parts = [np.asarray(res.results[c]["out"]) for c in range(N_CORES)]
    full = np.concatenate(parts, axis=1)[:, :OUT_F].astype(np.float32)
    return full, res


def kernel(**inputs) -> np.ndarray:
    full, _ = run(inputs, trace=False)
    return full


# revision 12
# speedup vs baseline: 1.3059x; 1.1720x over previous
"""Trainium2 Bass kernel for nn_CachedCompressedLinear.

out[16, 11008] = x[16, 4096] @ ((w_q - 128) * scale).T + bias

Sharding: column-parallel over 8 NeuronCores; each core computes a
[16, 1376] slice of the output (11008 = 8 * 1376).

The kernel is HBM-bound: the weight codes ship as 1 byte each
(w_q - 128 fits int8), 4x less traffic than the int32 input encoding.
The int8 -> bf16 decode is split DVE (tensor_scalar_add, ~0.88us per
k-tile) / ACT (activation Copy, ~1.44us) so both engines stay under
the ~17us DMA floor.  GpSimd gets NO tensor work: its Q7 software
loops hog the SBUF ports it shares with DVE and stall DVE for the
entire op (measured: each 20us GpSimd decode blocked DVE ~20us).
GpSimd only issues const/output DMAs (DGE trigger, harmless).

scale folds into x on the host (bf16 x, ~1.6e-3 rel err vs 2e-2
gate); bias folds into PSUM via one K=1 ones-row matmul per output
chunk, so the epilogue is a bare PSUM -> SBUF copy + DMA out.
"""

import sys

if "/opt/trn_rl_repo" not in sys.path:
    sys.path.insert(0, "/opt/trn_rl_repo")

import numpy as np
import ml_dtypes

IN_F = 4096
OUT_F = 11008
BATCH = 16
N_CORES = 8
O_PER = 1376  # out_features per core (11008 = 8 * 1376)
K_TILES = IN_F // 128  # 32
M = BATCH  # stationary columns (bf16 x, no hi/lo split)
CHUNKS = [(0, 512), (512, 512), (1024, 352)]  # o-chunks within 1376
PAIR = 2  # k-tiles per weight DMA

# per-k-tile decode engine: D=DVE, A=ACT; 20:12 matches measured
# per-tile decode times (1052ns DVE vs 1729ns ACT); A tiles sit
# second-in-pair so their higher latency gets an extra half-pair of
# slack before the PE needs them.
DECODE_PAT = "DADADADD" * 4

_BUILT = None


def _build():
    """Build the (SPMD, per-core) Bass program once."""
    import concourse.bass as bass
    import concourse.tile as tile
    from concourse import bacc, mybir

    dt = mybir.dt
    nc = bacc.Bacc("TRN2", target_bir_lowering=False, debug=False)

    w8 = nc.dram_tensor("w8", [128, K_TILES * O_PER], dt.int8, kind="ExternalInput")
    xt = nc.dram_tensor(
        "xt", [128, (K_TILES + 1) * M], dt.bfloat16, kind="ExternalInput"
    )
    bias_bf = nc.dram_tensor("bias_bf", [1, O_PER], dt.bfloat16, kind="ExternalInput")
    bias_f32 = nc.dram_tensor(
        "bias_f32", [BATCH, O_PER], dt.float32, kind="ExternalInput"
    )
    out = nc.dram_tensor("out", [BATCH, O_PER], dt.float32, kind="ExternalOutput")

    with tile.TileContext(nc) as tc:
        with (
            tc.tile_pool(name="consts", bufs=1) as consts,
            tc.tile_pool(name="w8p", bufs=6) as w8p,
            tc.tile_pool(name="wbf", bufs=8) as wbfp,
            tc.tile_pool(name="psum", bufs=1, space=bass.MemorySpace.PSUM) as psump,
            tc.tile_pool(name="outp", bufs=1) as outp,
        ):
            x_sb = consts.tile([128, (K_TILES + 1) * M], dt.bfloat16)
            nc.gpsimd.dma_start(x_sb[:], xt[:])
            bias_sb = consts.tile([1, O_PER], dt.bfloat16)
            nc.gpsimd.dma_start(bias_sb[:], bias_bf[:])
            bias_rep = consts.tile([BATCH, O_PER], dt.float32)
            nc.gpsimd.dma_start(bias_rep[:], bias_f32[:])

            psums = [
                psump.tile([M, w], dt.float32, name=f"ps{i}", tag=f"ps{i}")
                for i, (_, w) in enumerate(CHUNKS)
            ]
            ones_blk = x_sb[0:1, K_TILES * M : K_TILES * M + M]  # [1, 16]

            for g in range(K_TILES // PAIR):
                w8_t = w8p.tile([128, PAIR * O_PER], dt.int8, tag="w8_t")
                if g == 0:
                    # two single-tile DMAs so k0's decode starts as soon
                    # as the first 176KB lands (shorter pipeline fill)
                    for j in range(PAIR):
                        nc.sync.dma_start(
                            w8_t[:, j * O_PER : (j + 1) * O_PER],
                            w8[:][:, j * O_PER : (j + 1) * O_PER],
                        )
                else:
                    nc.sync.dma_start(
                        w8_t[:], w8[:][:, g * PAIR * O_PER : (g + 1) * PAIR * O_PER]
                    )
                for j in range(PAIR):
                    k = g * PAIR + j
                    wb_t = wbfp.tile([128, O_PER], dt.bfloat16, tag="wb_t")
                    src_ap = w8_t[:, j * O_PER : (j + 1) * O_PER]
                    if k == 0:
                        # split the first decode per chunk so the PE can
                        # start on chunk 0 ~0.7us earlier (pipeline fill)
                        for o, w in CHUNKS:
                            nc.vector.tensor_scalar_add(
                                wb_t[:, o : o + w], src_ap[:, o : o + w], 0.0
                            )
                    elif DECODE_PAT[k] == "D":
                        nc.vector.tensor_scalar_add(wb_t[:], src_ap, 0.0)
                    else:
                        nc.scalar.activation(
                            wb_t[:], src_ap, mybir.ActivationFunctionType.Copy
                        )
                    for i, (o, w) in enumerate(CHUNKS):
                        nc.tensor.matmul(
                            psums[i][:, :],
                            x_sb[:, k * M : (k + 1) * M],
                            wb_t[:, o : o + w],
                            start=(k == 0),
                            stop=(k == K_TILES - 1),
                        )
                    if k == 0:
                        # chunk-2 bias via K=1 matmul; chunks 0/1 get their
                        # bias in the epilogue add (keeps PE row count down)
                        o, w = CHUNKS[2]
                        nc.tensor.matmul(
                            psums[2][:, :],
                            ones_blk,
                            bias_sb[0:1, o : o + w],
                            start=False,
                            stop=False,
                        )

            out_q = [nc.gpsimd, nc.sync, nc.scalar]
            for i, (o, w) in enumerate(CHUNKS):
                ob = outp.tile([BATCH, w], dt.float32, name=f"ob{i}")
                if i == 2:
                    nc.scalar.activation(
                        ob[:], psums[i][:, :], mybir.ActivationFunctionType.Copy
                    )
                else:
                    nc.vector.tensor_add(
                        ob[:], psums[i][:, :], bias_rep[:, o : o + w]
                    )
                out_q[i].dma_start(out[:][:, o : o + w], ob[:])

    nc.compile()
    return nc


def _get_built():
    global _BUILT
    if _BUILT is None:
        _BUILT = _build()
    return _BUILT


def make_in_maps(x, w_q, scale, bias):
    """Host-side shard + layout prep. Returns per-core input dicts."""
    x = np.asarray(x, dtype=np.float32)
    w_q = np.asarray(w_q, dtype=np.int32)
    scale = np.asarray(scale, dtype=np.float32)
    bias = np.asarray(bias, dtype=np.float32)

    s = float(scale.reshape(-1)[0])
    xsT = np.ascontiguousarray((x * s).T).astype(ml_dtypes.bfloat16)  # [4096, 16]
    # SBUF layout [128, K_TILES*M]: partition p holds, for each k-tile t,
    # the stationary block row (t*128 + p); plus a ones block for bias.
    xt = np.zeros((128, (K_TILES + 1) * M), dtype=ml_dtypes.bfloat16)
    xt[:, : K_TILES * M] = (
        xsT.reshape(K_TILES, 128, M).transpose(1, 0, 2).reshape(128, K_TILES * M)
    )
    xt[0, K_TILES * M : K_TILES * M + M] = 1.0

    w8_full = (w_q.astype(np.int16) - 128).astype(np.int8)  # [11008, 4096]

    in_maps = []
    for c in range(N_CORES):
        sl = w8_full[c * O_PER : (c + 1) * O_PER].T  # [4096, 1376] int8
        w8_c = np.ascontiguousarray(
            sl.reshape(K_TILES, 128, O_PER).transpose(1, 0, 2).reshape(128, -1)
        )
        bias_c = np.ascontiguousarray(
            bias[c * O_PER : (c + 1) * O_PER].reshape(1, O_PER)
        )
        bias_rep_c = np.ascontiguousarray(
            np.broadcast_to(bias_c, (BATCH, O_PER))
        ).astype(np.float32)
        in_maps.append({
            "w8": w8_c,
            "xt": xt,
            "bias_bf": bias_c.astype(ml_dtypes.bfloat16),
            "bias_f32": bias_rep_c,
        })
    return in_maps


def run(inputs, trace=False):
    """Run on the 8 NeuronCores. Returns (full_output, BassKernelResults)."""
    from concourse.bass_utils import run_bass_kernel_spmd

    in_maps = make_in_maps(**inputs)
    nc = _get_built()
    res = run_bass_kernel_spmd(nc, in_maps, list(range(N_CORES)), trace=trace)
    parts = [np.asarray(res.results[c]["out"]) for c in range(N_CORES)]
    full = np.concatenate(parts, axis=1)[:, :OUT_F].astype(np.float32)
    return full, res


def kernel(**inputs) -> np.ndarray:
    full, _ = run(inputs, trace=False)
    return full
